# revision 6
# baseline (speedup 1.0000x reference)
"""SMEAR MoE layer (nn_MoELayer_SMEAR) Trainium2 Bass kernel, v2.

Problem: B=8, L=2048, D=1024, H=4096, E=8, fp32 in/out.
  logits = x @ router_w.T + router_b; probs = softmax(logits) * mask
  up = probs.sum(L) / clip(mask.sum(L), 1)            # [B, E]
  mW1 = up @ W1 ; mW2 = up @ W2 ; mb1 = up @ b1 ; mb2 = up @ b2
  out = relu(x @ mW1.T + mb1) @ mW2.T + mb2

Sharding (8 cores): dp=2 over B x tp=4 over H; host sums the 4 partial
outputs per dp-group.

v2 design (vs v1 which ran merge on PE and serialized phases):
- fp16 weight path end to end (x, W, merged W, hid, out partials).
  Numerically validated: max rel err ~6e-3 vs the 2e-2 budget (bf16 was
  1.7e-2+, too close).
- Weight merge runs on DVE + Pool(gpsimd), NOT on PE, overlapped with
  the MLP. W1 is merged in two H-half passes so L1 can start after the
  first pass; W2 merges under L1's shadow, in two D-half passes so early
  L2 output tiles unblock sooner.
- Merged weights round-trip DRAM in fp16, split into per-batch/per-half
  tensors so Tile's per-tensor DRAM dep tracking gives fine-grained
  readiness.
- MLP keeps each stationary tile for 4 back-to-back matmuls into 4 PSUM
  banks (LDWEIGHTS amortized; 8 banks double-buffer across groups).
- PE order L1(0) L1(1) L2(0) L1(2) L2(1) L1(3) L2(2) L2(3) so W2-merge
  latency hides while keeping only 2 hid buffers resident.
"""

import numpy as np

import concourse.bass as bass
import concourse.bacc as bacc
import concourse.mybir as mybir
import concourse.tile as tile
from concourse.bass_utils import run_bass_kernel_spmd
from concourse.masks import make_identity

P = 128
B, L, D, H, E = 8, 2048, 1024, 4096, 8
NB = 4          # batches per core
HS = H // 4     # h-shard width per core
DS = D // P     # 8 d-subtiles
HSUB = HS // P  # 8 h-subtiles in shard
OSUB = D // P   # 8 output subtiles
TCH = 512       # moving-dim chunk for matmuls
TC = L // TCH   # 4 chunks per batch
HHALF = HS // 2  # merge half-pass width

F32 = mybir.dt.float32
F16 = mybir.dt.float16
AF = mybir.ActivationFunctionType
ALU = mybir.AluOpType
AX = mybir.AxisListType

# merge unit engine split: unit index mod 4 in this set -> Pool engine
W1_POOL = ()
W2_POOL = ()

_CACHED_NC = None


def _build():
    nc = bacc.Bacc("TRN2", target_bir_lowering=False, debug=False)

    xT = nc.dram_tensor("xT", [NB, D, L], F16, kind="ExternalInput")
    maskT = nc.dram_tensor("maskT", [L, NB], F32, kind="ExternalInput")
    rwT = nc.dram_tensor("rwT", [D, E], F16, kind="ExternalInput")
    rb = nc.dram_tensor("rb", [E, 1], F32, kind="ExternalInput")
    W1T = nc.dram_tensor("W1T", [E, D, HS], F16, kind="ExternalInput")
    W2T = nc.dram_tensor("W2T", [E, HS, D], F16, kind="ExternalInput")
    b1T = nc.dram_tensor("b1T", [HS, E], F32, kind="ExternalInput")
    b2T = nc.dram_tensor("b2T", [D, E], F32, kind="ExternalInput")
    ownc = nc.dram_tensor("ownc", [NB, 1], F32, kind="ExternalInput")
    outp = nc.dram_tensor("outp", [NB, D, L], F16, kind="ExternalOutput")

    # merged weights, per (batch, half) for fine-grained DRAM deps
    mW1h = [[nc.dram_tensor(f"mW1_{b}_{h}", [D, HHALF], F16) for h in range(2)]
            for b in range(NB)]
    mW2h = [[nc.dram_tensor(f"mW2_{b}_{o}", [HS, HHALF], F16) for o in range(2)]
            for b in range(NB)]

    with tile.TileContext(nc) as tc:
        with tc.tile_pool(name="const", bufs=1) as const:
            ident = const.tile([P, P], F32)
            make_identity(nc, ident)
            ones_col = const.tile([P, 1], F32)
            nc.gpsimd.memset(ones_col[:], 1.0)
            ones_row = const.tile([1, P], F32)
            nc.gpsimd.memset(ones_row[:], 1.0)

            rwT_sb = const.tile([P, DS, E], F16)
            nc.sync.dma_start(rwT_sb[:], rwT.ap().rearrange("(s p) e -> p s e", p=P))
            rb_sb = const.tile([E, 1], F32)
            nc.sync.dma_start(rb_sb[:], rb.ap())
            maskT_sb = const.tile([P, L // P, NB], F32)
            nc.sync.dma_start(maskT_sb[:], maskT.ap().rearrange("(q p) b -> p q b", p=P))
            b1T_sb = const.tile([P, HSUB, E], F32)
            nc.sync.dma_start(b1T_sb[:], b1T.ap().rearrange("(s p) e -> p s e", p=P))
            b2T_sb = const.tile([P, OSUB, E], F32)
            nc.sync.dma_start(b2T_sb[:], b2T.ap().rearrange("(s p) e -> p s e", p=P))
            own_sb = const.tile([NB, 1], F32)
            nc.sync.dma_start(own_sb[:], ownc.ap())

            up_sb = const.tile([E, NB], F32)
            upT_sb = const.tile([NB, E], F32)
            upTo_sb = const.tile([NB, E], F32)
            up_bc = const.tile([P, NB, E], F32)
            upo_bc = const.tile([P, NB, E], F32)
            mb1_sb = const.tile([P, NB, HSUB], F32)
            mb2_sb = const.tile([P, NB, OSUB], F32)
            invbc_sb = const.tile([P, NB], F32)

            # ---------------- Phase B: router ----------------
            with tc.tile_pool(name="rpsum", bufs=1, space="PSUM") as rpsum, \
                 tc.tile_pool(name="rsb", bufs=6) as rsb, \
                 tc.tile_pool(name="xrt", bufs=4) as xrt, \
                 tc.tile_pool(name="lgp", bufs=2, space="PSUM") as lgp, \
                 tc.tile_pool(name="trp", bufs=3, space="PSUM") as trp, \
                 tc.tile_pool(name="upp", bufs=2, space="PSUM") as upp:

                # denominators: denom[b] = clip(sum_t mask, 1); invbc = 1/denom bcast
                mpart = rsb.tile([P, NB], F32)
                for b in range(NB):
                    nc.vector.tensor_reduce(
                        mpart[:, b:b + 1], maskT_sb[:, :, b], axis=AX.X, op=ALU.add)
                den_ps = rpsum.tile([NB, 1], F32, tag="rps")
                nc.tensor.matmul(den_ps[:], mpart[:], ones_col[:], start=True, stop=True)
                den_sb = rsb.tile([NB, 1], F32)
                nc.vector.tensor_scalar_max(den_sb[:], den_ps[:], 1.0)
                inv_sb = rsb.tile([NB, 1], F32)
                nc.vector.reciprocal(inv_sb[:], den_sb[:])
                invT_ps = rpsum.tile([1, NB], F32, tag="rps")
                nc.tensor.transpose(invT_ps[:], inv_sb[:], ident[:NB, :NB])
                invT_sb = rsb.tile([1, NB], F32)
                nc.vector.tensor_copy(invT_sb[:], invT_ps[:])
                invbc_ps = rpsum.tile([P, NB], F32, tag="rps")
                nc.tensor.matmul(invbc_ps[:], ones_row[:], invT_sb[:], start=True, stop=True)
                nc.vector.tensor_copy(invbc_sb[:], invbc_ps[:])

                NQ = TCH // P  # 4 transpose sub-chunks per 512 chunk
                for b in range(NB):
                    # maskS = mask * inv_denom for this b (free-dim broadcast)
                    maskS = rsb.tile([P, L // P], F32, tag="maskS")
                    nc.vector.tensor_tensor(
                        maskS[:], maskT_sb[:, :, b],
                        invbc_sb[:, b:b + 1].to_broadcast((P, L // P)), ALU.mult)
                    up_ps = upp.tile([E, 1], F32)
                    for t4 in range(TC):
                        xt = xrt.tile([P, DS, TCH], F16, tag="xrt")
                        nc.sync.dma_start(
                            xt[:],
                            xT.ap()[b].rearrange("(s p) t -> p s t", p=P)[
                                :, :, t4 * TCH:(t4 + 1) * TCH])
                        lg_ps = lgp.tile([E, TCH], F32)
                        for dsb in range(DS):
                            nc.tensor.matmul(lg_ps[:], rwT_sb[:, dsb], xt[:, dsb],
                                             start=(dsb == 0), stop=(dsb == DS - 1))
                        lgT = rsb.tile([E, TCH], F32, tag="lgT")
                        nc.scalar.activation(lgT[:], lg_ps[:], AF.Identity, bias=rb_sb[:])
                        # 4 transposes into one psum tile [P, 4*E]
                        tr_ps = trp.tile([P, NQ * E], F32)
                        for q in range(NQ):
                            nc.tensor.matmul(
                                tr_ps[:, q * E:(q + 1) * E],
                                lgT[:, q * P:(q + 1) * P], ident[:E, :E],
                                is_transpose=True,
                                start=(q == 0), stop=(q == NQ - 1))
                        pexp = rsb.tile([P, NQ, E], F32, tag="pexp")
                        nc.scalar.activation(pexp[:], tr_ps[:], AF.Exp)
                        s4 = rsb.tile([P, NQ], F32, tag="s4")
                        nc.vector.tensor_reduce(s4[:], pexp[:], axis=AX.X, op=ALU.add)
                        sr4 = rsb.tile([P, NQ], F32, tag="sr4")
                        nc.vector.reciprocal(sr4[:], s4[:])
                        r4 = rsb.tile([P, NQ], F32, tag="r4")
                        nc.vector.tensor_tensor(
                            r4[:], sr4[:], maskS[:, t4 * NQ:(t4 + 1) * NQ], ALU.mult)
                        for q in range(NQ):
                            nc.tensor.matmul(
                                up_ps[:], pexp[:, q], r4[:, q:q + 1],
                                start=(t4 == 0 and q == 0),
                                stop=(t4 == TC - 1 and q == NQ - 1))
                    nc.vector.tensor_copy(up_sb[:, b:b + 1], up_ps[:])

                # broadcast up across partitions; owner-masked copy for b2
                upT_ps = rpsum.tile([NB, E], F32, tag="rps")
                nc.tensor.transpose(upT_ps[:], up_sb[:], ident[:E, :E])
                nc.vector.tensor_copy(upT_sb[:], upT_ps[:])
                nc.vector.tensor_scalar_mul(upTo_sb[:], upT_sb[:], own_sb[:])
                for b in range(NB):
                    rowu = rsb.tile([1, E], F32, tag="rowu")
                    nc.sync.dma_start(rowu[:], upT_sb[b:b + 1, :])
                    rowo = rsb.tile([1, E], F32, tag="rowo")
                    nc.sync.dma_start(rowo[:], upTo_sb[b:b + 1, :])
                    bc_ps = rpsum.tile([P, E], F32, tag="rps")
                    nc.tensor.matmul(bc_ps[:], ones_row[:], rowu[:], start=True, stop=True)
                    nc.vector.tensor_copy(up_bc[:, b], bc_ps[:])
                    bo_ps = rpsum.tile([P, E], F32, tag="rps")
                    nc.tensor.matmul(bo_ps[:], ones_row[:], rowo[:], start=True, stop=True)
                    nc.vector.tensor_copy(upo_bc[:, b], bo_ps[:])

                # merged biases: mb1[b] = sum_e up[b,e] b1T[:,e]; mb2 owner-masked
                for b in range(NB):
                    nc.vector.tensor_scalar_mul(
                        mb1_sb[:, b], b1T_sb[:, :, 0], up_bc[:, b, 0:1])
                    nc.vector.tensor_scalar_mul(
                        mb2_sb[:, b], b2T_sb[:, :, 0], upo_bc[:, b, 0:1])
                    for e in range(1, E):
                        nc.vector.scalar_tensor_tensor(
                            mb1_sb[:, b], b1T_sb[:, :, e], up_bc[:, b, e:e + 1],
                            mb1_sb[:, b], ALU.mult, ALU.add)
                        nc.vector.scalar_tensor_tensor(
                            mb2_sb[:, b], b2T_sb[:, :, e], upo_bc[:, b, e:e + 1],
                            mb2_sb[:, b], ALU.mult, ALU.add)

            # -------- Phases C (merge, DVE+Pool) and D (MLP, PE) --------
            # Pools open together so SBUF regions are disjoint: no false
            # WAR deps between late merge ops and MLP tiles.
            with tc.tile_pool(name="rwp", bufs=2) as rwp, \
                 tc.tile_pool(name="mop", bufs=6) as mop, \
                 tc.tile_pool(name="xp", bufs=2) as xp, \
                 tc.tile_pool(name="hidp", bufs=2) as hidp, \
                 tc.tile_pool(name="wtp", bufs=3) as wtp, \
                 tc.tile_pool(name="osbp", bufs=4) as osbp, \
                 tc.tile_pool(name="mmp", bufs=2, space="PSUM") as mmp:

                # hoisted x prefetch for the first two batches
                x_tiles = {}
                for b in range(2):
                    xb = xp.tile([P, DS, L], F16, tag="x")
                    nc.sync.dma_start(
                        xb[:], xT.ap()[b].rearrange("(s p) t -> p s t", p=P))
                    x_tiles[b] = xb

                def merge_unit(eng, rw, b, dst):
                    mo = mop.tile([P, HHALF], F16, tag="mo")
                    eng.tensor_scalar_mul(mo[:], rw[:, 0], up_bc[:, b, 0:1])
                    for e in range(1, E):
                        eng.scalar_tensor_tensor(
                            mo[:], rw[:, e], up_bc[:, b, e:e + 1],
                            mo[:], ALU.mult, ALU.add)
                    nc.scalar.dma_start(dst, mo[:])

                # --- W1 merge: two H-half passes, chunk-major over dsb ---
                ucnt = 0
                for h in range(2):
                    for dsb in range(DS):
                        rw1 = rwp.tile([P, E, HHALF], F16, tag="rw1")
                        nc.sync.dma_start(
                            rw1[:],
                            W1T.ap()[:, dsb * P:(dsb + 1) * P,
                                     h * HHALF:(h + 1) * HHALF].rearrange(
                                "e p x -> p e x"))
                        for b in range(NB):
                            eng = nc.gpsimd if (ucnt % 4) in W1_POOL else nc.vector
                            merge_unit(eng, rw1, b,
                                       mW1h[b][h].ap()[dsb * P:(dsb + 1) * P, :])
                            ucnt += 1

                # --- W2 merge: two D(out)-half passes, chunk-major over hsb ---
                ucnt = 0
                for o in range(2):
                    for hsb in range(HSUB):
                        rw2 = rwp.tile([P, E, HHALF], F16, tag="rw2")
                        nc.scalar.dma_start(
                            rw2[:],
                            W2T.ap()[:, hsb * P:(hsb + 1) * P,
                                     o * HHALF:(o + 1) * HHALF].rearrange(
                                "e p x -> p e x"))
                        for b in range(NB):
                            eng = nc.gpsimd if (ucnt % 4) in W2_POOL else nc.vector
                            merge_unit(eng, rw2, b,
                                       mW2h[b][o].ap()[hsb * P:(hsb + 1) * P, :])
                            ucnt += 1

                # ---------------- Phase D: MLP ----------------
                hid_tiles = {}

                def l1(b):
                    if b in x_tiles:
                        xb = x_tiles[b]
                    else:
                        xb = xp.tile([P, DS, L], F16, tag="x")
                        nc.sync.dma_start(
                            xb[:], xT.ap()[b].rearrange("(s p) t -> p s t", p=P))
                    hidb = hidp.tile([P, HSUB, L], F16, tag="hid")
                    hid_tiles[b] = hidb
                    for hb in range(HSUB):
                        w1t = wtp.tile([P, DS, P], F16, tag="w1t")
                        nc.sync.dma_start(
                            w1t[:],
                            mW1h[b][hb // 4].ap().rearrange(
                                "(s p) x -> p s x", p=P)[
                                :, :, (hb % 4) * P:(hb % 4 + 1) * P])
                        pss = [mmp.tile([P, TCH], F32, tag=f"ps{q}",
                                        name=f"ps{q}")
                               for q in range(TC)]
                        for dsb in range(DS):
                            for q in range(TC):
                                nc.tensor.matmul(
                                    pss[q][:], w1t[:, dsb],
                                    xb[:, dsb, q * TCH:(q + 1) * TCH],
                                    start=(dsb == 0), stop=(dsb == DS - 1))
                        for q in range(TC):
                            nc.scalar.activation(
                                hidb[:, hb, q * TCH:(q + 1) * TCH], pss[q][:],
                                AF.Relu, bias=mb1_sb[:, b, hb:hb + 1])

                def l2(b):
                    hidb = hid_tiles[b]
                    for ob in range(OSUB):
                        w2t = wtp.tile([P, HSUB, P], F16, tag="w2t")
                        nc.sync.dma_start(
                            w2t[:],
                            mW2h[b][ob // 4].ap().rearrange(
                                "(s p) x -> p s x", p=P)[
                                :, :, (ob % 4) * P:(ob % 4 + 1) * P])
                        pss = [mmp.tile([P, TCH], F32, tag=f"ps{q}",
                                        name=f"ps{q}")
                               for q in range(TC)]
                        for hs in range(HSUB):
                            for q in range(TC):
                                nc.tensor.matmul(
                                    pss[q][:], w2t[:, hs],
                                    hidb[:, hs, q * TCH:(q + 1) * TCH],
                                    start=(hs == 0), stop=(hs == HSUB - 1))
                        for q in range(TC):
                            ot = osbp.tile([P, TCH], F16, tag="ot")
                            nc.scalar.activation(
                                ot[:], pss[q][:], AF.Identity,
                                bias=mb2_sb[:, b, ob:ob + 1])
                            nc.sync.dma_start(
                                outp.ap()[b, ob * P:(ob + 1) * P,
                                          q * TCH:(q + 1) * TCH], ot[:])

                l1(0)
                l1(1)
                l2(0)
                l1(2)
                l2(1)
                l1(3)
                l2(2)
                l2(3)

    nc.compile()
    return nc


def _get_nc():
    global _CACHED_NC
    if _CACHED_NC is None:
        _CACHED_NC = _build()
    return _CACHED_NC


def kernel(x, mask, router_w, router_b, W1, b1, W2, b2, _trace=False):
    x = np.asarray(x, np.float32)
    mask = np.asarray(mask, np.float32)
    router_w = np.asarray(router_w, np.float32)
    router_b = np.asarray(router_b, np.float32)
    W1 = np.asarray(W1, np.float32)
    b1 = np.asarray(b1, np.float32)
    W2 = np.asarray(W2, np.float32)
    b2 = np.asarray(b2, np.float32)

    nc = _get_nc()

    # host-side layout prep (sharding): transposes + fp16 casts
    xT_all = np.ascontiguousarray(x.transpose(0, 2, 1)).astype(np.float16)
    W1T_all = W1.transpose(0, 2, 1).astype(np.float16)    # [E, D, H]
    W2T_all = W2.transpose(0, 2, 1).astype(np.float16)    # [E, H, D]
    rwT = np.ascontiguousarray(router_w.T).astype(np.float16)  # [D, E]
    rbc = np.ascontiguousarray(router_b.reshape(E, 1))
    b1T_full = np.ascontiguousarray(b1.T)                 # [H, E]
    b2T = np.ascontiguousarray(b2.T)                      # [D, E]

    in_maps = []
    for c in range(8):
        g, r = c // 4, c % 4
        hs = slice(r * HS, (r + 1) * HS)
        own = np.zeros((NB, 1), np.float32)
        own[r, 0] = 1.0
        in_maps.append({
            "xT": xT_all[g * NB:(g + 1) * NB],
            "maskT": np.ascontiguousarray(mask[g * NB:(g + 1) * NB].T),
            "rwT": rwT,
            "rb": rbc,
            "W1T": np.ascontiguousarray(W1T_all[:, :, hs]),
            "W2T": np.ascontiguousarray(W2T_all[:, hs, :]),
            "b1T": np.ascontiguousarray(b1T_full[hs]),
            "b2T": b2T,
            "ownc": own,
        })

    res = run_bass_kernel_spmd(nc, in_maps, core_ids=list(range(8)),
                               trace=_trace)

    out = np.empty((B, L, D), np.float32)
    for g in range(2):
        acc = res.results[g * 4]["outp"].astype(np.float32)
        for r in range(1, 4):
            acc += res.results[g * 4 + r]["outp"].astype(np.float32)
        for j in range(NB):
            out[g * NB + j] = acc[j].T
    if _trace:
        return out, res
    return out


# revision 11
# speedup vs baseline: 1.2785x; 1.2785x over previous
"""SMEAR MoE layer (nn_MoELayer_SMEAR) Trainium2 Bass kernel, v2.

Problem: B=8, L=2048, D=1024, H=4096, E=8, fp32 in/out.
  logits = x @ router_w.T + router_b; probs = softmax(logits) * mask
  up = probs.sum(L) / clip(mask.sum(L), 1)            # [B, E]
  mW1 = up @ W1 ; mW2 = up @ W2 ; mb1 = up @ b1 ; mb2 = up @ b2
  out = relu(x @ mW1.T + mb1) @ mW2.T + mb2

Sharding (8 cores): dp=2 over B x tp=4 over H; host sums the 4 partial
outputs per dp-group.

v2 design (vs v1 which ran merge on PE and serialized phases):
- fp16 weight path end to end (x, W, merged W, hid, out partials).
  Numerically validated: max rel err ~6e-3 vs the 2e-2 budget (bf16 was
  1.7e-2+, too close).
- Weight merge runs on DVE + Pool(gpsimd), NOT on PE, overlapped with
  the MLP. W1 is merged in two H-half passes so L1 can start after the
  first pass; W2 merges under L1's shadow, in two D-half passes so early
  L2 output tiles unblock sooner.
- Merged weights round-trip DRAM in fp16, split into per-batch/per-half
  tensors so Tile's per-tensor DRAM dep tracking gives fine-grained
  readiness.
- MLP keeps each stationary tile for 4 back-to-back matmuls into 4 PSUM
  banks (LDWEIGHTS amortized; 8 banks double-buffer across groups).
- PE order L1(0) L1(1) L2(0) L1(2) L2(1) L1(3) L2(2) L2(3) so W2-merge
  latency hides while keeping only 2 hid buffers resident.
"""

import numpy as np

import concourse.bass as bass
import concourse.bacc as bacc
import concourse.mybir as mybir
import concourse.tile as tile
from concourse.bass_utils import run_bass_kernel_spmd
from concourse.masks import make_identity

P = 128
B, L, D, H, E = 8, 2048, 1024, 4096, 8
NB = 4          # batches per core
HS = H // 4     # h-shard width per core
DS = D // P     # 8 d-subtiles
HSUB = HS // P  # 8 h-subtiles in shard
OSUB = D // P   # 8 output subtiles
TCH = 512       # moving-dim chunk for matmuls
TC = L // TCH   # 4 chunks per batch
HHALF = HS // 2  # merge half-pass width

F32 = mybir.dt.float32
F16 = mybir.dt.float16
AF = mybir.ActivationFunctionType
ALU = mybir.AluOpType
AX = mybir.AxisListType

KMRG = 16           # weight rows merged per matmul (16 rows x 8 experts = 128)
NG = D * HS // (KMRG * HS)  # 64 row-groups per weight matrix

_CACHED_NC = None


def _build():
    nc = bacc.Bacc("TRN2", target_bir_lowering=False, debug=False)

    xT = nc.dram_tensor("xT", [NB, D, L], F16, kind="ExternalInput")
    maskT = nc.dram_tensor("maskT", [L, NB], F32, kind="ExternalInput")
    rwT = nc.dram_tensor("rwT", [D, E], F16, kind="ExternalInput")
    rb = nc.dram_tensor("rb", [E, 1], F32, kind="ExternalInput")
    # raw weights pre-grouped on host: [g, k*E+e, c] = WT[e, g*KMRG+k, c]
    W1G = nc.dram_tensor("W1G", [D // KMRG, P, HS], F16, kind="ExternalInput")
    W2G = nc.dram_tensor("W2G", [HS // KMRG, P, D], F16, kind="ExternalInput")
    b1T = nc.dram_tensor("b1T", [HS, E], F32, kind="ExternalInput")
    b2T = nc.dram_tensor("b2T", [D, E], F32, kind="ExternalInput")
    ownc = nc.dram_tensor("ownc", [NB, 1], F32, kind="ExternalInput")
    outp = nc.dram_tensor("outp", [NB, D, L], F16, kind="ExternalOutput")

    # merged weights, per batch for fine-grained DRAM deps
    mW1 = [nc.dram_tensor(f"mW1_{b}", [D, HS], F16) for b in range(NB)]
    mW2 = [nc.dram_tensor(f"mW2_{b}", [HS, D], F16) for b in range(NB)]

    with tile.TileContext(nc) as tc:
        with tc.tile_pool(name="const", bufs=1) as const:
            ident = const.tile([P, P], F32)
            make_identity(nc, ident)
            ones_col = const.tile([P, 1], F32)
            nc.gpsimd.memset(ones_col[:], 1.0)
            ones_row = const.tile([1, P], F32)
            nc.gpsimd.memset(ones_row[:], 1.0)

            rwT_sb = const.tile([P, DS, E], F16)
            nc.sync.dma_start(rwT_sb[:], rwT.ap().rearrange("(s p) e -> p s e", p=P))
            rb_sb = const.tile([E, 1], F32)
            nc.sync.dma_start(rb_sb[:], rb.ap())
            maskT_sb = const.tile([P, L // P, NB], F32)
            nc.sync.dma_start(maskT_sb[:], maskT.ap().rearrange("(q p) b -> p q b", p=P))
            b1T_sb = const.tile([P, HSUB, E], F32)
            nc.sync.dma_start(b1T_sb[:], b1T.ap().rearrange("(s p) e -> p s e", p=P))
            b2T_sb = const.tile([P, OSUB, E], F32)
            nc.sync.dma_start(b2T_sb[:], b2T.ap().rearrange("(s p) e -> p s e", p=P))
            own_sb = const.tile([NB, 1], F32)
            nc.sync.dma_start(own_sb[:], ownc.ap())

            up_sb = const.tile([E, NB], F32)
            upT_sb = const.tile([NB, E], F32)
            upTo_sb = const.tile([NB, E], F32)
            up_bc = const.tile([P, NB, E], F32)
            upo_bc = const.tile([P, NB, E], F32)
            mb1_sb = const.tile([P, NB, HSUB], F32)
            mb2_sb = const.tile([P, NB, OSUB], F32)
            invbc_sb = const.tile([P, NB], F32)
            # block-diag merge stationary: upblk[k*E+e, b*KMRG+k] = up[b, e]
            upblk = const.tile([P, NB * KMRG], F16)
            nc.gpsimd.memset(upblk[:], 0.0)

            # ---------------- Phase B: router ----------------
            with tc.tile_pool(name="rpsum", bufs=1, space="PSUM") as rpsum, \
                 tc.tile_pool(name="rsb", bufs=6) as rsb, \
                 tc.tile_pool(name="xrt", bufs=4) as xrt, \
                 tc.tile_pool(name="lgp", bufs=2, space="PSUM") as lgp, \
                 tc.tile_pool(name="trp", bufs=3, space="PSUM") as trp, \
                 tc.tile_pool(name="upp", bufs=2, space="PSUM") as upp:

                # denominators: denom[b] = clip(sum_t mask, 1); invbc = 1/denom bcast
                mpart = rsb.tile([P, NB], F32)
                for b in range(NB):
                    nc.vector.tensor_reduce(
                        mpart[:, b:b + 1], maskT_sb[:, :, b], axis=AX.X, op=ALU.add)
                den_ps = rpsum.tile([NB, 1], F32, tag="rps")
                nc.tensor.matmul(den_ps[:], mpart[:], ones_col[:], start=True, stop=True)
                den_sb = rsb.tile([NB, 1], F32)
                nc.vector.tensor_scalar_max(den_sb[:], den_ps[:], 1.0)
                inv_sb = rsb.tile([NB, 1], F32)
                nc.vector.reciprocal(inv_sb[:], den_sb[:])
                invT_ps = rpsum.tile([1, NB], F32, tag="rps")
                nc.tensor.transpose(invT_ps[:], inv_sb[:], ident[:NB, :NB])
                invT_sb = rsb.tile([1, NB], F32)
                nc.vector.tensor_copy(invT_sb[:], invT_ps[:])
                invbc_ps = rpsum.tile([P, NB], F32, tag="rps")
                nc.tensor.matmul(invbc_ps[:], ones_row[:], invT_sb[:], start=True, stop=True)
                nc.vector.tensor_copy(invbc_sb[:], invbc_ps[:])

                NQ = TCH // P  # 4 transpose sub-chunks per 512 chunk
                for b in range(NB):
                    # maskS = mask * inv_denom for this b (free-dim broadcast)
                    maskS = rsb.tile([P, L // P], F32, tag="maskS")
                    nc.vector.tensor_tensor(
                        maskS[:], maskT_sb[:, :, b],
                        invbc_sb[:, b:b + 1].to_broadcast((P, L // P)), ALU.mult)
                    up_ps = upp.tile([E, 1], F32)
                    for t4 in range(TC):
                        xt = xrt.tile([P, DS, TCH], F16, tag="xrt")
                        nc.sync.dma_start(
                            xt[:],
                            xT.ap()[b].rearrange("(s p) t -> p s t", p=P)[
                                :, :, t4 * TCH:(t4 + 1) * TCH])
                        lg_ps = lgp.tile([E, TCH], F32)
                        for dsb in range(DS):
                            nc.tensor.matmul(lg_ps[:], rwT_sb[:, dsb], xt[:, dsb],
                                             start=(dsb == 0), stop=(dsb == DS - 1))
                        lgT = rsb.tile([E, TCH], F32, tag="lgT")
                        nc.scalar.activation(lgT[:], lg_ps[:], AF.Identity, bias=rb_sb[:])
                        # 4 transposes into one psum tile [P, 4*E]
                        tr_ps = trp.tile([P, NQ * E], F32)
                        for q in range(NQ):
                            nc.tensor.matmul(
                                tr_ps[:, q * E:(q + 1) * E],
                                lgT[:, q * P:(q + 1) * P], ident[:E, :E],
                                is_transpose=True,
                                start=(q == 0), stop=(q == NQ - 1))
                        pexp = rsb.tile([P, NQ, E], F32, tag="pexp")
                        nc.scalar.activation(pexp[:], tr_ps[:], AF.Exp)
                        s4 = rsb.tile([P, NQ], F32, tag="s4")
                        nc.vector.tensor_reduce(s4[:], pexp[:], axis=AX.X, op=ALU.add)
                        sr4 = rsb.tile([P, NQ], F32, tag="sr4")
                        nc.vector.reciprocal(sr4[:], s4[:])
                        r4 = rsb.tile([P, NQ], F32, tag="r4")
                        nc.vector.tensor_tensor(
                            r4[:], sr4[:], maskS[:, t4 * NQ:(t4 + 1) * NQ], ALU.mult)
                        for q in range(NQ):
                            nc.tensor.matmul(
                                up_ps[:], pexp[:, q], r4[:, q:q + 1],
                                start=(t4 == 0 and q == 0),
                                stop=(t4 == TC - 1 and q == NQ - 1))
                    nc.vector.tensor_copy(up_sb[:, b:b + 1], up_ps[:])

                # broadcast up across partitions; owner-masked copy for b2
                upT_ps = rpsum.tile([NB, E], F32, tag="rps")
                nc.tensor.transpose(upT_ps[:], up_sb[:], ident[:E, :E])
                nc.vector.tensor_copy(upT_sb[:], upT_ps[:])
                nc.vector.tensor_scalar_mul(upTo_sb[:], upT_sb[:], own_sb[:])
                for b in range(NB):
                    rowu = rsb.tile([1, E], F32, tag="rowu")
                    nc.sync.dma_start(rowu[:], upT_sb[b:b + 1, :])
                    rowo = rsb.tile([1, E], F32, tag="rowo")
                    nc.sync.dma_start(rowo[:], upTo_sb[b:b + 1, :])
                    bc_ps = rpsum.tile([P, E], F32, tag="rps")
                    nc.tensor.matmul(bc_ps[:], ones_row[:], rowu[:], start=True, stop=True)
                    nc.vector.tensor_copy(up_bc[:, b], bc_ps[:])
                    bo_ps = rpsum.tile([P, E], F32, tag="rps")
                    nc.tensor.matmul(bo_ps[:], ones_row[:], rowo[:], start=True, stop=True)
                    nc.vector.tensor_copy(upo_bc[:, b], bo_ps[:])

                # block-diag stationary for the PE merge:
                # upblk[k*E+e, b*KMRG+k] = up[b, e].  DVE can't write at
                # partition offsets, so scatter with tiny SBUF->SBUF DMAs.
                uph_sb = rsb.tile([E, NB], F16)
                nc.vector.tensor_copy(uph_sb[:], up_sb[:])
                for k in range(KMRG):
                    nc.sync.dma_start(
                        upblk[k * E:(k + 1) * E].rearrange(
                            "p (b k2) -> p b k2", k2=KMRG)[:, :, k],
                        uph_sb[:])

                # merged biases: mb1[b] = sum_e up[b,e] b1T[:,e]; mb2 owner-masked
                for b in range(NB):
                    nc.vector.tensor_scalar_mul(
                        mb1_sb[:, b], b1T_sb[:, :, 0], up_bc[:, b, 0:1])
                    nc.vector.tensor_scalar_mul(
                        mb2_sb[:, b], b2T_sb[:, :, 0], upo_bc[:, b, 0:1])
                    for e in range(1, E):
                        nc.vector.scalar_tensor_tensor(
                            mb1_sb[:, b], b1T_sb[:, :, e], up_bc[:, b, e:e + 1],
                            mb1_sb[:, b], ALU.mult, ALU.add)
                        nc.vector.scalar_tensor_tensor(
                            mb2_sb[:, b], b2T_sb[:, :, e], upo_bc[:, b, e:e + 1],
                            mb2_sb[:, b], ALU.mult, ALU.add)

            # ---- Phases C (merge, PE block-diag matmuls) and D (MLP) ----
            # Pools open together so SBUF regions are disjoint: no false
            # WAR deps between late merge ops and MLP tiles.
            NSB = NG // 8         # 8 superblocks of 8 row-groups
            with tc.tile_pool(name="rwp", bufs=6) as rwp, \
                 tc.tile_pool(name="mop", bufs=2) as mop, \
                 tc.tile_pool(name="xp", bufs=2) as xp, \
                 tc.tile_pool(name="hidp", bufs=2) as hidp, \
                 tc.tile_pool(name="wtp", bufs=3) as wtp, \
                 tc.tile_pool(name="osbp", bufs=4) as osbp, \
                 tc.tile_pool(name="mmp", bufs=2, space="PSUM") as mmp:

                # hoisted x prefetch for the first two batches
                x_tiles = {}
                for b in range(2):
                    xb = xp.tile([P, DS, L], F16, tag="x", name="xb")
                    nc.sync.dma_start(
                        xb[:], xT.ap()[b].rearrange("(s p) t -> p s t", p=P))
                    x_tiles[b] = xb

                def merge_w(raw, dst):
                    """dst[b][r, c] = sum_e up[b,e] raw[e, r, c]; raw rows
                    grouped 16 at a time across PE partitions.  Drains on
                    DVE, writes on ACT, loads on SP: each stays under the
                    raw-stream pacing so the merge is DMA-bound only."""
                    rawv = raw.ap()
                    ncol = raw.shape[2] // HHALF
                    for sb in range(NSB):
                        mos = [mop.tile([NB * KMRG, 8, HHALF], F16,
                                        tag=f"mo{c}", name="mos")
                               for c in range(ncol)]
                        for gg in range(8):
                            g = sb * 8 + gg
                            rw = rwp.tile([P, HS], F16, tag="rw", name="rw")
                            nc.sync.dma_start(rw[:], rawv[g])
                            for c in range(ncol):
                                ps = mmp.tile([P, TCH], F32,
                                              tag=f"ps{(gg * ncol + c) % 4}",
                                              name="psm")
                                nc.tensor.matmul(
                                    ps[:NB * KMRG, :], upblk[:],
                                    rw[:, c * HHALF:(c + 1) * HHALF],
                                    start=True, stop=True)
                                nc.vector.tensor_copy(
                                    mos[c][:, gg, :], ps[:NB * KMRG, :])
                        rows = 8 * KMRG
                        for b in range(NB):
                            for c in range(ncol):
                                nc.scalar.dma_start(
                                    dst[b].ap()[sb * rows:(sb + 1) * rows,
                                                c * HHALF:(c + 1) * HHALF]
                                    .rearrange("(gg k) h -> k gg h", k=KMRG),
                                    mos[c][b * KMRG:(b + 1) * KMRG])

                # ---------------- Phase D: MLP ----------------
                hid_tiles = {}

                def l1(b):
                    if b in x_tiles:
                        xb = x_tiles[b]
                    else:
                        xb = xp.tile([P, DS, L], F16, tag="x", name="xb")
                        nc.sync.dma_start(
                            xb[:], xT.ap()[b].rearrange("(s p) t -> p s t", p=P))
                    hidb = hidp.tile([P, HSUB, L], F16, tag="hid", name="hidb")
                    hid_tiles[b] = hidb
                    for hb in range(HSUB):
                        w1t = wtp.tile([P, DS, P], F16, tag="w1t", name="w1t")
                        nc.sync.dma_start(
                            w1t[:],
                            mW1[b].ap().rearrange("(s p) x -> p s x", p=P)[
                                :, :, hb * P:(hb + 1) * P])
                        pss = [mmp.tile([P, TCH], F32, tag=f"ps{q}",
                                        name=f"ps{q}")
                               for q in range(TC)]
                        for dsb in range(DS):
                            for q in range(TC):
                                nc.tensor.matmul(
                                    pss[q][:], w1t[:, dsb],
                                    xb[:, dsb, q * TCH:(q + 1) * TCH],
                                    start=(dsb == 0), stop=(dsb == DS - 1))
                        for q in range(TC):
                            nc.scalar.activation(
                                hidb[:, hb, q * TCH:(q + 1) * TCH], pss[q][:],
                                AF.Relu, bias=mb1_sb[:, b, hb:hb + 1])

                def l2(b):
                    hidb = hid_tiles[b]
                    for ob in range(OSUB):
                        w2t = wtp.tile([P, HSUB, P], F16, tag="w2t", name="w2t")
                        nc.sync.dma_start(
                            w2t[:],
                            mW2[b].ap().rearrange("(s p) x -> p s x", p=P)[
                                :, :, ob * P:(ob + 1) * P])
                        pss = [mmp.tile([P, TCH], F32, tag=f"ps{q}",
                                        name=f"ps{q}")
                               for q in range(TC)]
                        for hs in range(HSUB):
                            for q in range(TC):
                                nc.tensor.matmul(
                                    pss[q][:], w2t[:, hs],
                                    hidb[:, hs, q * TCH:(q + 1) * TCH],
                                    start=(hs == 0), stop=(hs == HSUB - 1))
                        for q in range(TC):
                            ot = osbp.tile([P, TCH], F16, tag="ot", name="ot")
                            nc.vector.tensor_scalar_add(
                                ot[:], pss[q][:], mb2_sb[:, b, ob:ob + 1])
                            nc.sync.dma_start(
                                outp.ap()[b, ob * P:(ob + 1) * P,
                                          q * TCH:(q + 1) * TCH], ot[:])

                merge_w(W1G, mW1)
                l1(0)
                merge_w(W2G, mW2)
                l1(1)
                l2(0)
                l1(2)
                l2(1)
                l1(3)
                l2(2)
                l2(3)

    nc.compile()
    return nc


def _get_nc():
    global _CACHED_NC
    if _CACHED_NC is None:
        _CACHED_NC = _build()
    return _CACHED_NC


def kernel(x, mask, router_w, router_b, W1, b1, W2, b2, _trace=False):
    x = np.asarray(x, np.float32)
    mask = np.asarray(mask, np.float32)
    router_w = np.asarray(router_w, np.float32)
    router_b = np.asarray(router_b, np.float32)
    W1 = np.asarray(W1, np.float32)
    b1 = np.asarray(b1, np.float32)
    W2 = np.asarray(W2, np.float32)
    b2 = np.asarray(b2, np.float32)

    nc = _get_nc()

    # host-side layout prep (sharding): transposes + fp16 casts
    xT_all = np.ascontiguousarray(x.transpose(0, 2, 1)).astype(np.float16)
    W1T_all = W1.transpose(0, 2, 1).astype(np.float16)    # [E, D, H]
    W2T_all = W2.transpose(0, 2, 1).astype(np.float16)    # [E, H, D]
    rwT = np.ascontiguousarray(router_w.T).astype(np.float16)  # [D, E]
    rbc = np.ascontiguousarray(router_b.reshape(E, 1))
    b1T_full = np.ascontiguousarray(b1.T)                 # [H, E]
    b2T = np.ascontiguousarray(b2.T)                      # [D, E]

    in_maps = []
    for c in range(8):
        g, r = c // 4, c % 4
        hs = slice(r * HS, (r + 1) * HS)
        own = np.zeros((NB, 1), np.float32)
        own[r, 0] = 1.0
        w1g = W1T_all[:, :, hs].reshape(E, D // 16, 16, HS).transpose(
            1, 2, 0, 3).reshape(D // 16, 128, HS)
        w2g = W2T_all[:, hs, :].reshape(E, HS // 16, 16, D).transpose(
            1, 2, 0, 3).reshape(HS // 16, 128, D)
        in_maps.append({
            "xT": xT_all[g * NB:(g + 1) * NB],
            "maskT": np.ascontiguousarray(mask[g * NB:(g + 1) * NB].T),
            "rwT": rwT,
            "rb": rbc,
            "W1G": np.ascontiguousarray(w1g),
            "W2G": np.ascontiguousarray(w2g),
            "b1T": np.ascontiguousarray(b1T_full[hs]),
            "b2T": b2T,
            "ownc": own,
        })

    res = run_bass_kernel_spmd(nc, in_maps, core_ids=list(range(8)),
                               trace=_trace)

    out = np.empty((B, L, D), np.float32)
    for g in range(2):
        acc = res.results[g * 4]["outp"].astype(np.float32)
        for r in range(1, 4):
            acc += res.results[g * 4 + r]["outp"].astype(np.float32)
        for j in range(NB):
            out[g * NB + j] = acc[j].T
    if _trace:
        return out, res
    return out


# revision 12
# speedup vs baseline: 1.3289x; 1.0395x over previous
"""SMEAR MoE layer (nn_MoELayer_SMEAR) Trainium2 Bass kernel, v2.

Problem: B=8, L=2048, D=1024, H=4096, E=8, fp32 in/out.
  logits = x @ router_w.T + router_b; probs = softmax(logits) * mask
  up = probs.sum(L) / clip(mask.sum(L), 1)            # [B, E]
  mW1 = up @ W1 ; mW2 = up @ W2 ; mb1 = up @ b1 ; mb2 = up @ b2
  out = relu(x @ mW1.T + mb1) @ mW2.T + mb2

Sharding (8 cores): dp=2 over B x tp=4 over H; host sums the 4 partial
outputs per dp-group.

v2 design (vs v1 which ran merge on PE and serialized phases):
- fp16 weight path end to end (x, W, merged W, hid, out partials).
  Numerically validated: max rel err ~6e-3 vs the 2e-2 budget (bf16 was
  1.7e-2+, too close).
- Weight merge runs on DVE + Pool(gpsimd), NOT on PE, overlapped with
  the MLP. W1 is merged in two H-half passes so L1 can start after the
  first pass; W2 merges under L1's shadow, in two D-half passes so early
  L2 output tiles unblock sooner.
- Merged weights round-trip DRAM in fp16, split into per-batch/per-half
  tensors so Tile's per-tensor DRAM dep tracking gives fine-grained
  readiness.
- MLP keeps each stationary tile for 4 back-to-back matmuls into 4 PSUM
  banks (LDWEIGHTS amortized; 8 banks double-buffer across groups).
- PE order L1(0) L1(1) L2(0) L1(2) L2(1) L1(3) L2(2) L2(3) so W2-merge
  latency hides while keeping only 2 hid buffers resident.
"""

import numpy as np

import concourse.bass as bass
import concourse.bacc as bacc
import concourse.mybir as mybir
import concourse.tile as tile
from concourse.bass_utils import run_bass_kernel_spmd
from concourse.masks import make_identity

P = 128
B, L, D, H, E = 8, 2048, 1024, 4096, 8
NB = 4          # batches per core
HS = H // 4     # h-shard width per core
DS = D // P     # 8 d-subtiles
HSUB = HS // P  # 8 h-subtiles in shard
OSUB = D // P   # 8 output subtiles
TCH = 512       # moving-dim chunk for matmuls
TC = L // TCH   # 4 chunks per batch
HHALF = HS // 2  # merge half-pass width

F32 = mybir.dt.float32
F16 = mybir.dt.float16
AF = mybir.ActivationFunctionType
ALU = mybir.AluOpType
AX = mybir.AxisListType

KMRG = 16           # weight rows merged per matmul (16 rows x 8 experts = 128)
NG = D * HS // (KMRG * HS)  # 64 row-groups per weight matrix

_CACHED_NC = None


def _build():
    nc = bacc.Bacc("TRN2", target_bir_lowering=False, debug=False)

    xT = nc.dram_tensor("xT", [NB, D, L], F16, kind="ExternalInput")
    maskT = nc.dram_tensor("maskT", [L, NB], F32, kind="ExternalInput")
    rwT = nc.dram_tensor("rwT", [D, E], F16, kind="ExternalInput")
    rb = nc.dram_tensor("rb", [E, 1], F32, kind="ExternalInput")
    # raw weights pre-grouped on host: [g, k*E+e, c] = WT[e, g*KMRG+k, c]
    W1G = nc.dram_tensor("W1G", [D // KMRG, P, HS], F16, kind="ExternalInput")
    W2G = nc.dram_tensor("W2G", [HS // KMRG, P, D], F16, kind="ExternalInput")
    b1T = nc.dram_tensor("b1T", [HS, E], F32, kind="ExternalInput")
    b2T = nc.dram_tensor("b2T", [D, E], F32, kind="ExternalInput")
    ownc = nc.dram_tensor("ownc", [NB, 1], F32, kind="ExternalInput")
    outp = nc.dram_tensor("outp", [NB, D, L], F16, kind="ExternalOutput")

    # merged weights, per batch for fine-grained DRAM deps
    mW1 = [nc.dram_tensor(f"mW1_{b}", [D, HS], F16) for b in range(NB)]
    mW2 = [nc.dram_tensor(f"mW2_{b}", [HS, D], F16) for b in range(NB)]

    with tile.TileContext(nc) as tc:
        with tc.tile_pool(name="const", bufs=1) as const:
            ident = const.tile([P, P], F32)
            make_identity(nc, ident)
            ones_col = const.tile([P, 1], F32)
            nc.gpsimd.memset(ones_col[:], 1.0)
            ones_row = const.tile([1, P], F32)
            nc.gpsimd.memset(ones_row[:], 1.0)

            rwT_sb = const.tile([P, DS, E], F16)
            nc.sync.dma_start(rwT_sb[:], rwT.ap().rearrange("(s p) e -> p s e", p=P))
            rb_sb = const.tile([E, 1], F32)
            nc.sync.dma_start(rb_sb[:], rb.ap())
            maskT_sb = const.tile([P, L // P, NB], F32)
            nc.sync.dma_start(maskT_sb[:], maskT.ap().rearrange("(q p) b -> p q b", p=P))
            b1T_sb = const.tile([P, HSUB, E], F32)
            nc.sync.dma_start(b1T_sb[:], b1T.ap().rearrange("(s p) e -> p s e", p=P))
            b2T_sb = const.tile([P, OSUB, E], F32)
            nc.sync.dma_start(b2T_sb[:], b2T.ap().rearrange("(s p) e -> p s e", p=P))
            own_sb = const.tile([NB, 1], F32)
            nc.sync.dma_start(own_sb[:], ownc.ap())

            up_sb = const.tile([E, NB], F32)
            upT_sb = const.tile([NB, E], F32)
            upTo_sb = const.tile([NB, E], F32)
            up_bc = const.tile([P, NB, E], F32)
            upo_bc = const.tile([P, NB, E], F32)
            mb1_sb = const.tile([P, NB, HSUB], F32)
            mb2_sb = const.tile([P, NB, OSUB], F32)
            invbc_sb = const.tile([P, NB], F32)
            # block-diag merge stationary: upblk[k*E+e, b*KMRG+k] = up[b, e]
            upblk = const.tile([P, NB * KMRG], F16)
            nc.gpsimd.memset(upblk[:], 0.0)

            # ---------------- Phase B: router ----------------
            with tc.tile_pool(name="rpsum", bufs=1, space="PSUM") as rpsum, \
                 tc.tile_pool(name="rsb", bufs=6) as rsb, \
                 tc.tile_pool(name="xrt", bufs=4) as xrt, \
                 tc.tile_pool(name="lgp", bufs=2, space="PSUM") as lgp, \
                 tc.tile_pool(name="trp", bufs=3, space="PSUM") as trp, \
                 tc.tile_pool(name="upp", bufs=2, space="PSUM") as upp:

                # denominators: denom[b] = clip(sum_t mask, 1); invbc = 1/denom bcast
                mpart = rsb.tile([P, NB], F32)
                for b in range(NB):
                    nc.vector.tensor_reduce(
                        mpart[:, b:b + 1], maskT_sb[:, :, b], axis=AX.X, op=ALU.add)
                den_ps = rpsum.tile([NB, 1], F32, tag="rps")
                nc.tensor.matmul(den_ps[:], mpart[:], ones_col[:], start=True, stop=True)
                den_sb = rsb.tile([NB, 1], F32)
                nc.vector.tensor_scalar_max(den_sb[:], den_ps[:], 1.0)
                inv_sb = rsb.tile([NB, 1], F32)
                nc.vector.reciprocal(inv_sb[:], den_sb[:])
                invT_ps = rpsum.tile([1, NB], F32, tag="rps")
                nc.tensor.transpose(invT_ps[:], inv_sb[:], ident[:NB, :NB])
                invT_sb = rsb.tile([1, NB], F32)
                nc.vector.tensor_copy(invT_sb[:], invT_ps[:])
                invbc_ps = rpsum.tile([P, NB], F32, tag="rps")
                nc.tensor.matmul(invbc_ps[:], ones_row[:], invT_sb[:], start=True, stop=True)
                nc.vector.tensor_copy(invbc_sb[:], invbc_ps[:])

                NQ = TCH // P  # 4 transpose sub-chunks per 512 chunk
                for b in range(NB):
                    # maskS = mask * inv_denom for this b (free-dim broadcast)
                    maskS = rsb.tile([P, L // P], F32, tag="maskS")
                    nc.vector.tensor_tensor(
                        maskS[:], maskT_sb[:, :, b],
                        invbc_sb[:, b:b + 1].to_broadcast((P, L // P)), ALU.mult)
                    up_ps = upp.tile([E, 1], F32)
                    for t4 in range(TC):
                        xt = xrt.tile([P, DS, TCH], F16, tag="xrt")
                        nc.sync.dma_start(
                            xt[:],
                            xT.ap()[b].rearrange("(s p) t -> p s t", p=P)[
                                :, :, t4 * TCH:(t4 + 1) * TCH])
                        lg_ps = lgp.tile([E, TCH], F32)
                        for dsb in range(DS):
                            nc.tensor.matmul(lg_ps[:], rwT_sb[:, dsb], xt[:, dsb],
                                             start=(dsb == 0), stop=(dsb == DS - 1))
                        lgT = rsb.tile([E, TCH], F32, tag="lgT")
                        nc.scalar.activation(lgT[:], lg_ps[:], AF.Identity, bias=rb_sb[:])
                        # 4 transposes into one psum tile [P, 4*E]
                        tr_ps = trp.tile([P, NQ * E], F32)
                        for q in range(NQ):
                            nc.tensor.matmul(
                                tr_ps[:, q * E:(q + 1) * E],
                                lgT[:, q * P:(q + 1) * P], ident[:E, :E],
                                is_transpose=True,
                                start=(q == 0), stop=(q == NQ - 1))
                        pexp = rsb.tile([P, NQ, E], F32, tag="pexp")
                        nc.scalar.activation(pexp[:], tr_ps[:], AF.Exp)
                        s4 = rsb.tile([P, NQ], F32, tag="s4")
                        nc.vector.tensor_reduce(s4[:], pexp[:], axis=AX.X, op=ALU.add)
                        sr4 = rsb.tile([P, NQ], F32, tag="sr4")
                        nc.vector.reciprocal(sr4[:], s4[:])
                        r4 = rsb.tile([P, NQ], F32, tag="r4")
                        nc.vector.tensor_tensor(
                            r4[:], sr4[:], maskS[:, t4 * NQ:(t4 + 1) * NQ], ALU.mult)
                        for q in range(NQ):
                            nc.tensor.matmul(
                                up_ps[:], pexp[:, q], r4[:, q:q + 1],
                                start=(t4 == 0 and q == 0),
                                stop=(t4 == TC - 1 and q == NQ - 1))
                    nc.vector.tensor_copy(up_sb[:, b:b + 1], up_ps[:])

                # broadcast up across partitions; owner-masked copy for b2
                upT_ps = rpsum.tile([NB, E], F32, tag="rps")
                nc.tensor.transpose(upT_ps[:], up_sb[:], ident[:E, :E])
                nc.vector.tensor_copy(upT_sb[:], upT_ps[:])
                nc.vector.tensor_scalar_mul(upTo_sb[:], upT_sb[:], own_sb[:])
                for b in range(NB):
                    rowu = rsb.tile([1, E], F32, tag="rowu")
                    nc.sync.dma_start(rowu[:], upT_sb[b:b + 1, :])
                    rowo = rsb.tile([1, E], F32, tag="rowo")
                    nc.sync.dma_start(rowo[:], upTo_sb[b:b + 1, :])
                    bc_ps = rpsum.tile([P, E], F32, tag="rps")
                    nc.tensor.matmul(bc_ps[:], ones_row[:], rowu[:], start=True, stop=True)
                    nc.vector.tensor_copy(up_bc[:, b], bc_ps[:])
                    bo_ps = rpsum.tile([P, E], F32, tag="rps")
                    nc.tensor.matmul(bo_ps[:], ones_row[:], rowo[:], start=True, stop=True)
                    nc.vector.tensor_copy(upo_bc[:, b], bo_ps[:])

                # block-diag stationary for the PE merge:
                # upblk[k*E+e, b*KMRG+k] = up[b, e].  DVE can't write at
                # partition offsets, so scatter with tiny SBUF->SBUF DMAs.
                uph_sb = rsb.tile([E, NB], F16)
                nc.vector.tensor_copy(uph_sb[:], up_sb[:])
                for k in range(KMRG):
                    nc.scalar.dma_start(
                        upblk[k * E:(k + 1) * E].rearrange(
                            "p (b k2) -> p b k2", k2=KMRG)[:, :, k],
                        uph_sb[:])

                # merged biases: mb1[b] = sum_e up[b,e] b1T[:,e]; mb2 owner-masked
                for b in range(NB):
                    nc.vector.tensor_scalar_mul(
                        mb1_sb[:, b], b1T_sb[:, :, 0], up_bc[:, b, 0:1])
                    nc.vector.tensor_scalar_mul(
                        mb2_sb[:, b], b2T_sb[:, :, 0], upo_bc[:, b, 0:1])
                    for e in range(1, E):
                        nc.vector.scalar_tensor_tensor(
                            mb1_sb[:, b], b1T_sb[:, :, e], up_bc[:, b, e:e + 1],
                            mb1_sb[:, b], ALU.mult, ALU.add)
                        nc.vector.scalar_tensor_tensor(
                            mb2_sb[:, b], b2T_sb[:, :, e], upo_bc[:, b, e:e + 1],
                            mb2_sb[:, b], ALU.mult, ALU.add)

            # ---- Phases C (merge, PE block-diag matmuls) and D (MLP) ----
            # Pools open together so SBUF regions are disjoint: no false
            # WAR deps between late merge ops and MLP tiles.
            NSB = NG // 8         # 8 superblocks of 8 row-groups
            with tc.tile_pool(name="rwp", bufs=6) as rwp, \
                 tc.tile_pool(name="mop", bufs=2) as mop, \
                 tc.tile_pool(name="xp", bufs=2) as xp, \
                 tc.tile_pool(name="hidp", bufs=2) as hidp, \
                 tc.tile_pool(name="wtp", bufs=3) as wtp, \
                 tc.tile_pool(name="osbp", bufs=4) as osbp, \
                 tc.tile_pool(name="mmp", bufs=2, space="PSUM") as mmp:

                # hoisted x prefetch for the first two batches
                x_tiles = {}
                for b in range(2):
                    xb = xp.tile([P, DS, L], F16, tag="x", name="xb")
                    nc.sync.dma_start(
                        xb[:], xT.ap()[b].rearrange("(s p) t -> p s t", p=P))
                    x_tiles[b] = xb

                def merge_w(raw, dst, dr):
                    """dst[b][r, c] = sum_e up[b,e] raw[e, r, c]; raw rows
                    grouped 16 at a time across PE partitions.  The two
                    column halves of a group share one PSUM bank (partitions
                    0-63 / 64-127) so a single full-width drain empties it;
                    drains alternate ACT/DVE to stay under stream pacing."""
                    rawv = raw.ap()
                    for sb in range(NSB):
                        mos = mop.tile([P, 8, HHALF], F16, tag="mo",
                                       name="mos")
                        for gg in range(8):
                            g = sb * 8 + gg
                            rw = rwp.tile([P, HS], F16, tag="rw", name="rw")
                            nc.sync.dma_start(rw[:], rawv[g])
                            ps = mmp.tile([P, TCH], F32, tag=f"ps{gg % 4}",
                                          name="psm")
                            for c in range(2):
                                nc.tensor.matmul(
                                    ps[c * 64:(c + 1) * 64, :], upblk[:],
                                    rw[:, c * HHALF:(c + 1) * HHALF],
                                    start=True, stop=True)
                            if dr[0] % 2 == 0:
                                nc.scalar.activation(mos[:, gg, :], ps[:],
                                                     AF.Identity)
                            else:
                                nc.vector.tensor_copy(mos[:, gg, :], ps[:])
                            dr[0] += 1
                        rows = 8 * KMRG
                        for b in range(NB):
                            for c in range(2):
                                nc.scalar.dma_start(
                                    dst[b].ap()[sb * rows:(sb + 1) * rows,
                                                c * HHALF:(c + 1) * HHALF]
                                    .rearrange("(gg k) h -> k gg h", k=KMRG),
                                    mos[c * 64 + b * KMRG:
                                        c * 64 + (b + 1) * KMRG])

                # ---------------- Phase D: MLP ----------------
                hid_tiles = {}

                def l1(b):
                    if b in x_tiles:
                        xb = x_tiles[b]
                    else:
                        xb = xp.tile([P, DS, L], F16, tag="x", name="xb")
                        nc.sync.dma_start(
                            xb[:], xT.ap()[b].rearrange("(s p) t -> p s t", p=P))
                    hidb = hidp.tile([P, HSUB, L], F16, tag="hid", name="hidb")
                    hid_tiles[b] = hidb
                    for hb in range(HSUB):
                        w1t = wtp.tile([P, DS, P], F16, tag="w1t", name="w1t")
                        nc.sync.dma_start(
                            w1t[:],
                            mW1[b].ap().rearrange("(s p) x -> p s x", p=P)[
                                :, :, hb * P:(hb + 1) * P])
                        pss = [mmp.tile([P, TCH], F32, tag=f"ps{q}",
                                        name=f"ps{q}")
                               for q in range(TC)]
                        for dsb in range(DS):
                            for q in range(TC):
                                nc.tensor.matmul(
                                    pss[q][:], w1t[:, dsb],
                                    xb[:, dsb, q * TCH:(q + 1) * TCH],
                                    start=(dsb == 0), stop=(dsb == DS - 1))
                        for q in range(TC):
                            nc.scalar.activation(
                                hidb[:, hb, q * TCH:(q + 1) * TCH], pss[q][:],
                                AF.Relu, bias=mb1_sb[:, b, hb:hb + 1])

                def l2(b):
                    hidb = hid_tiles[b]
                    for ob in range(OSUB):
                        w2t = wtp.tile([P, HSUB, P], F16, tag="w2t", name="w2t")
                        nc.sync.dma_start(
                            w2t[:],
                            mW2[b].ap().rearrange("(s p) x -> p s x", p=P)[
                                :, :, ob * P:(ob + 1) * P])
                        pss = [mmp.tile([P, TCH], F32, tag=f"ps{q}",
                                        name=f"ps{q}")
                               for q in range(TC)]
                        for hs in range(HSUB):
                            for q in range(TC):
                                nc.tensor.matmul(
                                    pss[q][:], w2t[:, hs],
                                    hidb[:, hs, q * TCH:(q + 1) * TCH],
                                    start=(hs == 0), stop=(hs == HSUB - 1))
                        for q in range(TC):
                            ot = osbp.tile([P, TCH], F16, tag="ot", name="ot")
                            nc.vector.tensor_scalar_add(
                                ot[:], pss[q][:], mb2_sb[:, b, ob:ob + 1])
                            nc.sync.dma_start(
                                outp.ap()[b, ob * P:(ob + 1) * P,
                                          q * TCH:(q + 1) * TCH], ot[:])

                drc = [0]
                merge_w(W1G, mW1, drc)
                l1(0)
                merge_w(W2G, mW2, drc)
                l1(1)
                l2(0)
                l1(2)
                l2(1)
                l1(3)
                l2(2)
                l2(3)

    nc.compile()
    return nc


def _get_nc():
    global _CACHED_NC
    if _CACHED_NC is None:
        _CACHED_NC = _build()
    return _CACHED_NC


def kernel(x, mask, router_w, router_b, W1, b1, W2, b2, _trace=False):
    x = np.asarray(x, np.float32)
    mask = np.asarray(mask, np.float32)
    router_w = np.asarray(router_w, np.float32)
    router_b = np.asarray(router_b, np.float32)
    W1 = np.asarray(W1, np.float32)
    b1 = np.asarray(b1, np.float32)
    W2 = np.asarray(W2, np.float32)
    b2 = np.asarray(b2, np.float32)

    nc = _get_nc()

    # host-side layout prep (sharding): transposes + fp16 casts
    xT_all = np.ascontiguousarray(x.transpose(0, 2, 1)).astype(np.float16)
    W1T_all = W1.transpose(0, 2, 1).astype(np.float16)    # [E, D, H]
    W2T_all = W2.transpose(0, 2, 1).astype(np.float16)    # [E, H, D]
    rwT = np.ascontiguousarray(router_w.T).astype(np.float16)  # [D, E]
    rbc = np.ascontiguousarray(router_b.reshape(E, 1))
    b1T_full = np.ascontiguousarray(b1.T)                 # [H, E]
    b2T = np.ascontiguousarray(b2.T)                      # [D, E]

    in_maps = []
    for c in range(8):
        g, r = c // 4, c % 4
        hs = slice(r * HS, (r + 1) * HS)
        own = np.zeros((NB, 1), np.float32)
        own[r, 0] = 1.0
        w1g = W1T_all[:, :, hs].reshape(E, D // 16, 16, HS).transpose(
            1, 2, 0, 3).reshape(D // 16, 128, HS)
        w2g = W2T_all[:, hs, :].reshape(E, HS // 16, 16, D).transpose(
            1, 2, 0, 3).reshape(HS // 16, 128, D)
        in_maps.append({
            "xT": xT_all[g * NB:(g + 1) * NB],
            "maskT": np.ascontiguousarray(mask[g * NB:(g + 1) * NB].T),
            "rwT": rwT,
            "rb": rbc,
            "W1G": np.ascontiguousarray(w1g),
            "W2G": np.ascontiguousarray(w2g),
            "b1T": np.ascontiguousarray(b1T_full[hs]),
            "b2T": b2T,
            "ownc": own,
        })

    res = run_bass_kernel_spmd(nc, in_maps, core_ids=list(range(8)),
                               trace=_trace)

    out = np.empty((B, L, D), np.float32)
    for g in range(2):
        acc = res.results[g * 4]["outp"].astype(np.float32)
        for r in range(1, 4):
            acc += res.results[g * 4 + r]["outp"].astype(np.float32)
        for j in range(NB):
            out[g * NB + j] = acc[j].T
    if _trace:
        return out, res
    return out


# revision 14
# speedup vs baseline: 1.3642x; 1.0265x over previous
"""SMEAR MoE layer (nn_MoELayer_SMEAR) Trainium2 Bass kernel, v2.

Problem: B=8, L=2048, D=1024, H=4096, E=8, fp32 in/out.
  logits = x @ router_w.T + router_b; probs = softmax(logits) * mask
  up = probs.sum(L) / clip(mask.sum(L), 1)            # [B, E]
  mW1 = up @ W1 ; mW2 = up @ W2 ; mb1 = up @ b1 ; mb2 = up @ b2
  out = relu(x @ mW1.T + mb1) @ mW2.T + mb2

Sharding (8 cores): dp=2 over B x tp=4 over H; host sums the 4 partial
outputs per dp-group.

v2 design (vs v1 which ran merge on PE and serialized phases):
- fp16 weight path end to end (x, W, merged W, hid, out partials).
  Numerically validated: max rel err ~6e-3 vs the 2e-2 budget (bf16 was
  1.7e-2+, too close).
- Weight merge runs on DVE + Pool(gpsimd), NOT on PE, overlapped with
  the MLP. W1 is merged in two H-half passes so L1 can start after the
  first pass; W2 merges under L1's shadow, in two D-half passes so early
  L2 output tiles unblock sooner.
- Merged weights round-trip DRAM in fp16, split into per-batch/per-half
  tensors so Tile's per-tensor DRAM dep tracking gives fine-grained
  readiness.
- MLP keeps each stationary tile for 4 back-to-back matmuls into 4 PSUM
  banks (LDWEIGHTS amortized; 8 banks double-buffer across groups).
- PE order L1(0) L1(1) L2(0) L1(2) L2(1) L1(3) L2(2) L2(3) so W2-merge
  latency hides while keeping only 2 hid buffers resident.
"""

import numpy as np

import concourse.bass as bass
import concourse.bacc as bacc
import concourse.mybir as mybir
import concourse.tile as tile
from concourse.bass_utils import run_bass_kernel_spmd
from concourse.masks import make_identity

P = 128
B, L, D, H, E = 8, 2048, 1024, 4096, 8
NB = 4          # batches per core
HS = H // 4     # h-shard width per core
DS = D // P     # 8 d-subtiles
HSUB = HS // P  # 8 h-subtiles in shard
OSUB = D // P   # 8 output subtiles
TCH = 512       # moving-dim chunk for matmuls
TC = L // TCH   # 4 chunks per batch
HHALF = HS // 2  # merge half-pass width

F32 = mybir.dt.float32
F16 = mybir.dt.float16
AF = mybir.ActivationFunctionType
ALU = mybir.AluOpType
AX = mybir.AxisListType

KMRG = 16           # weight rows merged per matmul (16 rows x 8 experts = 128)
NG = D * HS // (KMRG * HS)  # 64 row-groups per weight matrix

_CACHED_NC = None


def _build():
    nc = bacc.Bacc("TRN2", target_bir_lowering=False, debug=False)

    xT = nc.dram_tensor("xT", [NB, D, L], F16, kind="ExternalInput")
    maskT = nc.dram_tensor("maskT", [L, NB], F32, kind="ExternalInput")
    rwT = nc.dram_tensor("rwT", [D, E], F16, kind="ExternalInput")
    rb = nc.dram_tensor("rb", [E, 1], F32, kind="ExternalInput")
    # raw weights pre-grouped on host: [g, k*E+e, c] = WT[e, g*KMRG+k, c]
    W1G = nc.dram_tensor("W1G", [D // KMRG, P, HS], F16, kind="ExternalInput")
    W2G = nc.dram_tensor("W2G", [HS // KMRG, P, D], F16, kind="ExternalInput")
    b1T = nc.dram_tensor("b1T", [HS, E], F32, kind="ExternalInput")
    b2T = nc.dram_tensor("b2T", [D, E], F32, kind="ExternalInput")
    ownc = nc.dram_tensor("ownc", [NB, 1], F32, kind="ExternalInput")
    outp = nc.dram_tensor("outp", [NB, D, L], F16, kind="ExternalOutput")

    # merged weights, per batch for fine-grained DRAM deps
    mW1 = [nc.dram_tensor(f"mW1_{b}", [D, HS], F16) for b in range(NB)]
    mW2 = [nc.dram_tensor(f"mW2_{b}", [HS, D], F16) for b in range(NB)]

    with tile.TileContext(nc) as tc:
        with tc.tile_pool(name="const", bufs=1) as const:
            ident = const.tile([P, P], F32)
            make_identity(nc, ident)
            ones_col = const.tile([P, 1], F32)
            nc.gpsimd.memset(ones_col[:], 1.0)
            ones_row = const.tile([1, P], F32)
            nc.gpsimd.memset(ones_row[:], 1.0)

            rwT_sb = const.tile([P, DS, E], F16)
            nc.sync.dma_start(rwT_sb[:], rwT.ap().rearrange("(s p) e -> p s e", p=P))
            rb_sb = const.tile([E, 1], F32)
            nc.sync.dma_start(rb_sb[:], rb.ap())
            maskT_sb = const.tile([P, L // P, NB], F32)
            nc.sync.dma_start(maskT_sb[:], maskT.ap().rearrange("(q p) b -> p q b", p=P))
            b1T_sb = const.tile([P, HSUB, E], F32)
            nc.sync.dma_start(b1T_sb[:], b1T.ap().rearrange("(s p) e -> p s e", p=P))
            b2T_sb = const.tile([P, OSUB, E], F32)
            nc.sync.dma_start(b2T_sb[:], b2T.ap().rearrange("(s p) e -> p s e", p=P))
            own_sb = const.tile([NB, 1], F32)
            nc.sync.dma_start(own_sb[:], ownc.ap())

            up_sb = const.tile([E, NB], F32)
            upT_sb = const.tile([NB, E], F32)
            upTo_sb = const.tile([NB, E], F32)
            up_bc = const.tile([P, NB, E], F32)
            upo_bc = const.tile([P, NB, E], F32)
            mb1_sb = const.tile([P, NB, HSUB], F32)
            mb2_sb = const.tile([P, NB, OSUB], F32)
            invbc_sb = const.tile([P, NB], F32)
            # block-diag merge stationary: upblk[k*E+e, b*KMRG+k] = up[b, e]
            upblk = const.tile([P, NB * KMRG], F16)
            nc.gpsimd.memset(upblk[:], 0.0)

            # ---------------- Phase B: router ----------------
            with tc.tile_pool(name="rpsum", bufs=1, space="PSUM") as rpsum, \
                 tc.tile_pool(name="rsb", bufs=6) as rsb, \
                 tc.tile_pool(name="xrt", bufs=4) as xrt, \
                 tc.tile_pool(name="lgp", bufs=2, space="PSUM") as lgp, \
                 tc.tile_pool(name="trp", bufs=3, space="PSUM") as trp, \
                 tc.tile_pool(name="upp", bufs=2, space="PSUM") as upp:

                # denominators: denom[b] = clip(sum_t mask, 1); invbc = 1/denom bcast
                mpart = rsb.tile([P, NB], F32)
                for b in range(NB):
                    nc.vector.tensor_reduce(
                        mpart[:, b:b + 1], maskT_sb[:, :, b], axis=AX.X, op=ALU.add)
                den_ps = rpsum.tile([NB, 1], F32, tag="rps")
                nc.tensor.matmul(den_ps[:], mpart[:], ones_col[:], start=True, stop=True)
                den_sb = rsb.tile([NB, 1], F32)
                nc.vector.tensor_scalar_max(den_sb[:], den_ps[:], 1.0)
                inv_sb = rsb.tile([NB, 1], F32)
                nc.vector.reciprocal(inv_sb[:], den_sb[:])
                invT_ps = rpsum.tile([1, NB], F32, tag="rps")
                nc.tensor.transpose(invT_ps[:], inv_sb[:], ident[:NB, :NB])
                invT_sb = rsb.tile([1, NB], F32)
                nc.vector.tensor_copy(invT_sb[:], invT_ps[:])
                invbc_ps = rpsum.tile([P, NB], F32, tag="rps")
                nc.tensor.matmul(invbc_ps[:], ones_row[:], invT_sb[:], start=True, stop=True)
                nc.vector.tensor_copy(invbc_sb[:], invbc_ps[:])

                NQ = TCH // P  # 4 transpose sub-chunks per 512 chunk
                for b in range(NB):
                    # maskS = mask * inv_denom for this b (free-dim broadcast)
                    maskS = rsb.tile([P, L // P], F32, tag="maskS")
                    nc.vector.tensor_tensor(
                        maskS[:], maskT_sb[:, :, b],
                        invbc_sb[:, b:b + 1].to_broadcast((P, L // P)), ALU.mult)
                    up_ps = upp.tile([E, 1], F32)
                    pend = []  # software pipeline: up-matmuls lag one chunk
                    for t4 in range(TC):
                        xt = xrt.tile([P, DS, TCH], F16, tag="xrt")
                        nc.sync.dma_start(
                            xt[:],
                            xT.ap()[b].rearrange("(s p) t -> p s t", p=P)[
                                :, :, t4 * TCH:(t4 + 1) * TCH])
                        lg_ps = lgp.tile([E, TCH], F32)
                        for dsb in range(DS):
                            nc.tensor.matmul(lg_ps[:], rwT_sb[:, dsb], xt[:, dsb],
                                             start=(dsb == 0), stop=(dsb == DS - 1))
                        lgT = rsb.tile([E, TCH], F32, tag="lgT")
                        nc.scalar.activation(lgT[:], lg_ps[:], AF.Identity, bias=rb_sb[:])
                        # 4 transposes into one psum tile [P, 4*E]
                        tr_ps = trp.tile([P, NQ * E], F32)
                        for q in range(NQ):
                            nc.tensor.matmul(
                                tr_ps[:, q * E:(q + 1) * E],
                                lgT[:, q * P:(q + 1) * P], ident[:E, :E],
                                is_transpose=True,
                                start=(q == 0), stop=(q == NQ - 1))
                        pexp = rsb.tile([P, NQ, E], F32, tag="pexp")
                        nc.scalar.activation(pexp[:], tr_ps[:], AF.Exp)
                        s4 = rsb.tile([P, NQ], F32, tag="s4")
                        nc.vector.tensor_reduce(s4[:], pexp[:], axis=AX.X, op=ALU.add)
                        sr4 = rsb.tile([P, NQ], F32, tag="sr4")
                        nc.vector.reciprocal(sr4[:], s4[:])
                        r4 = rsb.tile([P, NQ], F32, tag="r4")
                        nc.vector.tensor_tensor(
                            r4[:], sr4[:], maskS[:, t4 * NQ:(t4 + 1) * NQ], ALU.mult)
                        pend.append((pexp, r4, t4))
                        if t4 > 0:
                            pp, rr, tt = pend.pop(0)
                            for q in range(NQ):
                                nc.tensor.matmul(
                                    up_ps[:], pp[:, q], rr[:, q:q + 1],
                                    start=(tt == 0 and q == 0), stop=False)
                    pp, rr, tt = pend.pop(0)
                    for q in range(NQ):
                        nc.tensor.matmul(
                            up_ps[:], pp[:, q], rr[:, q:q + 1],
                            start=False, stop=(q == NQ - 1))
                    nc.vector.tensor_copy(up_sb[:, b:b + 1], up_ps[:])

                # broadcast up across partitions; owner-masked copy for b2
                upT_ps = rpsum.tile([NB, E], F32, tag="rps")
                nc.tensor.transpose(upT_ps[:], up_sb[:], ident[:E, :E])
                nc.vector.tensor_copy(upT_sb[:], upT_ps[:])
                nc.vector.tensor_scalar_mul(upTo_sb[:], upT_sb[:], own_sb[:])
                for b in range(NB):
                    rowu = rsb.tile([1, E], F32, tag="rowu")
                    nc.sync.dma_start(rowu[:], upT_sb[b:b + 1, :])
                    rowo = rsb.tile([1, E], F32, tag="rowo")
                    nc.sync.dma_start(rowo[:], upTo_sb[b:b + 1, :])
                    bc_ps = rpsum.tile([P, E], F32, tag="rps")
                    nc.tensor.matmul(bc_ps[:], ones_row[:], rowu[:], start=True, stop=True)
                    nc.vector.tensor_copy(up_bc[:, b], bc_ps[:])
                    bo_ps = rpsum.tile([P, E], F32, tag="rps")
                    nc.tensor.matmul(bo_ps[:], ones_row[:], rowo[:], start=True, stop=True)
                    nc.vector.tensor_copy(upo_bc[:, b], bo_ps[:])

                # block-diag stationary for the PE merge:
                # upblk[k*E+e, b*KMRG+k] = up[b, e].  DVE can't write at
                # partition offsets, so scatter with tiny SBUF->SBUF DMAs.
                uph_sb = rsb.tile([E, NB], F16)
                nc.vector.tensor_copy(uph_sb[:], up_sb[:])
                for k in range(KMRG):
                    eng = nc.scalar if k % 2 else nc.sync
                    eng.dma_start(
                        upblk[k * E:(k + 1) * E].rearrange(
                            "p (b k2) -> p b k2", k2=KMRG)[:, :, k],
                        uph_sb[:])

                # merged biases: mb1[b] = sum_e up[b,e] b1T[:,e]; mb2 owner-masked
                for b in range(NB):
                    nc.vector.tensor_scalar_mul(
                        mb1_sb[:, b], b1T_sb[:, :, 0], up_bc[:, b, 0:1])
                    nc.vector.tensor_scalar_mul(
                        mb2_sb[:, b], b2T_sb[:, :, 0], upo_bc[:, b, 0:1])
                    for e in range(1, E):
                        nc.vector.scalar_tensor_tensor(
                            mb1_sb[:, b], b1T_sb[:, :, e], up_bc[:, b, e:e + 1],
                            mb1_sb[:, b], ALU.mult, ALU.add)
                        nc.vector.scalar_tensor_tensor(
                            mb2_sb[:, b], b2T_sb[:, :, e], upo_bc[:, b, e:e + 1],
                            mb2_sb[:, b], ALU.mult, ALU.add)

            # ---- Phases C (merge, PE block-diag matmuls) and D (MLP) ----
            # Pools open together so SBUF regions are disjoint: no false
            # WAR deps between late merge ops and MLP tiles.
            NSB = NG // 16        # 4 superblocks of 16 row-groups
            with tc.tile_pool(name="rwp", bufs=6) as rwp, \
                 tc.tile_pool(name="mop", bufs=2) as mop, \
                 tc.tile_pool(name="xp", bufs=2) as xp, \
                 tc.tile_pool(name="hidp", bufs=2) as hidp, \
                 tc.tile_pool(name="wtp", bufs=3) as wtp, \
                 tc.tile_pool(name="osbp", bufs=4) as osbp, \
                 tc.tile_pool(name="mmp", bufs=2, space="PSUM") as mmp:

                # hoisted x prefetch for the first two batches
                def load_x(b):
                    xb = xp.tile([P, DS, L], F16, tag="x", name="xb")
                    xv = xT.ap()[b].rearrange("(s p) t -> p s t", p=P)
                    nc.sync.dma_start(xb[:, 0:4], xv[:, 0:4])
                    nc.scalar.dma_start(xb[:, 4:8], xv[:, 4:8])
                    return xb

                x_tiles = {b: load_x(b) for b in range(2)}

                def merge_w(raw, dst, dr):
                    """dst[b][r, c] = sum_e up[b,e] raw[e, r, c]; raw rows
                    grouped 16 at a time across PE partitions.  The two
                    column halves of a group share one PSUM bank (partitions
                    0-63 / 64-127) so a single full-width drain empties it;
                    drains alternate ACT/DVE to stay under stream pacing."""
                    rawv = raw.ap()
                    for sb in range(NSB):
                        mos = mop.tile([P, 16, HHALF], F16, tag="mo",
                                       name="mos")
                        for gg in range(16):
                            g = sb * 16 + gg
                            rw = rwp.tile([P, HS], F16, tag="rw", name="rw")
                            ldq = nc.sync if gg % 2 else nc.scalar
                            ldq.dma_start(rw[:], rawv[g])
                            ps = mmp.tile([P, TCH], F32, tag=f"ps{gg % 4}",
                                          name="psm")
                            for c in range(2):
                                nc.tensor.matmul(
                                    ps[c * 64:(c + 1) * 64, :], upblk[:],
                                    rw[:, c * HHALF:(c + 1) * HHALF],
                                    start=True, stop=True)
                            if dr[0] % 2 == 0:
                                nc.scalar.activation(mos[:, gg, :], ps[:],
                                                     AF.Identity)
                            else:
                                nc.vector.tensor_copy(mos[:, gg, :], ps[:])
                            dr[0] += 1
                        rows = 16 * KMRG
                        for b in range(NB):
                            for c in range(2):
                                nc.sync.dma_start(
                                    dst[b].ap()[sb * rows:(sb + 1) * rows,
                                                c * HHALF:(c + 1) * HHALF]
                                    .rearrange("(gg k) h -> k gg h", k=KMRG),
                                    mos[c * 64 + b * KMRG:
                                        c * 64 + (b + 1) * KMRG])

                # ---------------- Phase D: MLP ----------------
                hid_tiles = {}

                def l1(b):
                    xb = x_tiles[b] if b in x_tiles else load_x(b)
                    hidb = hidp.tile([P, HSUB, L], F16, tag="hid", name="hidb")
                    hid_tiles[b] = hidb
                    for hb in range(HSUB):
                        w1t = wtp.tile([P, DS, P], F16, tag="w1t", name="w1t")
                        nc.sync.dma_start(
                            w1t[:],
                            mW1[b].ap().rearrange("(s p) x -> p s x", p=P)[
                                :, :, hb * P:(hb + 1) * P])
                        pss = [mmp.tile([P, TCH], F32, tag=f"ps{q}",
                                        name=f"ps{q}")
                               for q in range(TC)]
                        for dsb in range(DS):
                            for q in range(TC):
                                nc.tensor.matmul(
                                    pss[q][:], w1t[:, dsb],
                                    xb[:, dsb, q * TCH:(q + 1) * TCH],
                                    start=(dsb == 0), stop=(dsb == DS - 1))
                        for q in range(TC):
                            nc.scalar.activation(
                                hidb[:, hb, q * TCH:(q + 1) * TCH], pss[q][:],
                                AF.Relu, bias=mb1_sb[:, b, hb:hb + 1])

                def l2(b):
                    hidb = hid_tiles[b]
                    for ob in range(OSUB):
                        w2t = wtp.tile([P, HSUB, P], F16, tag="w2t", name="w2t")
                        nc.sync.dma_start(
                            w2t[:],
                            mW2[b].ap().rearrange("(s p) x -> p s x", p=P)[
                                :, :, ob * P:(ob + 1) * P])
                        pss = [mmp.tile([P, TCH], F32, tag=f"ps{q}",
                                        name=f"ps{q}")
                               for q in range(TC)]
                        for hs in range(HSUB):
                            for q in range(TC):
                                nc.tensor.matmul(
                                    pss[q][:], w2t[:, hs],
                                    hidb[:, hs, q * TCH:(q + 1) * TCH],
                                    start=(hs == 0), stop=(hs == HSUB - 1))
                        for q in range(TC):
                            ot = osbp.tile([P, TCH], F16, tag="ot", name="ot")
                            nc.vector.tensor_scalar_add(
                                ot[:], pss[q][:], mb2_sb[:, b, ob:ob + 1])
                            nc.sync.dma_start(
                                outp.ap()[b, ob * P:(ob + 1) * P,
                                          q * TCH:(q + 1) * TCH], ot[:])

                drc = [0]
                merge_w(W1G, mW1, drc)
                l1(0)
                merge_w(W2G, mW2, drc)
                l1(1)
                l2(0)
                l1(2)
                l2(1)
                l1(3)
                l2(2)
                l2(3)

    nc.compile()
    return nc


def _get_nc():
    global _CACHED_NC
    if _CACHED_NC is None:
        _CACHED_NC = _build()
    return _CACHED_NC


def kernel(x, mask, router_w, router_b, W1, b1, W2, b2, _trace=False):
    x = np.asarray(x, np.float32)
    mask = np.asarray(mask, np.float32)
    router_w = np.asarray(router_w, np.float32)
    router_b = np.asarray(router_b, np.float32)
    W1 = np.asarray(W1, np.float32)
    b1 = np.asarray(b1, np.float32)
    W2 = np.asarray(W2, np.float32)
    b2 = np.asarray(b2, np.float32)

    nc = _get_nc()

    # host-side layout prep (sharding): transposes + fp16 casts
    xT_all = np.ascontiguousarray(x.transpose(0, 2, 1)).astype(np.float16)
    W1T_all = W1.transpose(0, 2, 1).astype(np.float16)    # [E, D, H]
    W2T_all = W2.transpose(0, 2, 1).astype(np.float16)    # [E, H, D]
    rwT = np.ascontiguousarray(router_w.T).astype(np.float16)  # [D, E]
    rbc = np.ascontiguousarray(router_b.reshape(E, 1))
    b1T_full = np.ascontiguousarray(b1.T)                 # [H, E]
    b2T = np.ascontiguousarray(b2.T)                      # [D, E]

    in_maps = []
    for c in range(8):
        g, r = c // 4, c % 4
        hs = slice(r * HS, (r + 1) * HS)
        own = np.zeros((NB, 1), np.float32)
        own[r, 0] = 1.0
        w1g = W1T_all[:, :, hs].reshape(E, D // 16, 16, HS).transpose(
            1, 2, 0, 3).reshape(D // 16, 128, HS)
        w2g = W2T_all[:, hs, :].reshape(E, HS // 16, 16, D).transpose(
            1, 2, 0, 3).reshape(HS // 16, 128, D)
        in_maps.append({
            "xT": xT_all[g * NB:(g + 1) * NB],
            "maskT": np.ascontiguousarray(mask[g * NB:(g + 1) * NB].T),
            "rwT": rwT,
            "rb": rbc,
            "W1G": np.ascontiguousarray(w1g),
            "W2G": np.ascontiguousarray(w2g),
            "b1T": np.ascontiguousarray(b1T_full[hs]),
            "b2T": b2T,
            "ownc": own,
        })

    res = run_bass_kernel_spmd(nc, in_maps, core_ids=list(range(8)),
                               trace=_trace)

    out = np.empty((B, L, D), np.float32)
    for g in range(2):
        acc = res.results[g * 4]["outp"].astype(np.float32)
        for r in range(1, 4):
            acc += res.results[g * 4 + r]["outp"].astype(np.float32)
        for j in range(NB):
            out[g * NB + j] = acc[j].T
    if _trace:
        return out, res
    return out
